# revision 69
# baseline (speedup 1.0000x reference)
"""MoE top-2 routing kernel for Trainium2 (8 NeuronCores, pair-dispatch).

Key algebraic trick: the reference combine is an UNWEIGHTED mean, so
  out = 0.5*(x@Wa + be_a + x@Wb + be_b) = x @ (0.5*(Wa+Wb)) + 0.5*(be_a+be_b).
Dispatching tokens by their (unordered) top-2 expert PAIR and pre-summing
the 45 possible pair matrices on the host means ONE matmul per token
instead of two: half the PE streams and half the x/out HBM traffic of a
per-expert dispatch.

All device traffic is bf16 (x, pair weights, outputs; fp32 PSUM
accumulate), halving HBM bytes again. Measured rel-err of the full
pipeline ~3e-3 vs the fp32 reference (gate: 2e-2).

Layout per core (SPMD — all 8 cores run one compiled program):
  - `s` weight slots, slot j sized F_j 128-token tiles; each slot holds
    one pair matrix [1024,1024] bf16 = 2MB, DMA'd in a single transfer.
  - token tiles packed in super-tiles of B=2 tiles -> 0.5MB x loads /
    out stores for DMA efficiency.
  - matmul: x tile chunk [128k,128m] stationary, pair weights streamed
    [128k, 512n] x2 into a [128,1024] fp32 PSUM; DVE copies+casts to a
    bf16 out super-tile.
Host combines: out[token] = partial[pos[token]] + 0.5*(be_a + be_b).
"""

import os
from contextlib import ExitStack

import ml_dtypes
import numpy as np

import concourse.bass as bass
import concourse.mybir as mybir
import concourse.tile as tile
from concourse import bacc
from concourse.bass_utils import run_bass_kernel_spmd

N = 8192
D = 1024
E = 10
TOP_K = 2
P = 128
KC = D // P  # 8 contraction chunks of 128
NCORES = 8
B = 2  # token tiles per super-tile (DMA batch)
NPAIR = E * (E - 1) // 2  # 45

_last_results = None  # stash for test harness (exec_time_ns etc.)


def _route(x, Wr, br):
    """Top-2 expert ids per token, replicating reference ops exactly."""
    import jax
    import jax.numpy as jnp

    logits = jnp.asarray(x) @ jnp.asarray(Wr).T + jnp.asarray(br)
    probs = jax.nn.softmax(logits, axis=-1)
    _, idx = jax.lax.top_k(probs, TOP_K)
    return np.asarray(idx)


def _pack(tiles_per_pair):
    """Choose slot sizes (shared by all cores, SPMD) and assign pair
    tile-pieces to (core, slot) cells. Returns (sizes, assign): assign is
    a list of (pair, core, slot, n_tiles)."""
    active = [(t, e) for e, t in enumerate(tiles_per_pair) if t > 0]
    total = sum(t for t, _ in active)
    tmax = max(t for t, _ in active)
    ncells_min = len(active)  # each piece needs its own cell

    def try_shape(sizes):
        cells = sorted(
            [(sz, c, j) for j, sz in enumerate(sizes) for c in range(NCORES)],
            reverse=True,
        )
        rem = sorted(active, reverse=True)
        assign = []
        ci = 0
        while rem:
            rem.sort(reverse=True)
            t, e = rem.pop(0)
            if ci >= len(cells):
                return None
            sz, c, j = cells[ci]
            ci += 1
            if sz == 0:
                return None
            take = min(t, sz)
            assign.append((e, c, j, take))
            if t - take > 0:
                rem.append((t - take, e))
        return assign

    best = None
    base = -(-total // NCORES)  # ceil
    smin = max(1, -(-ncells_min // NCORES))
    for s in range(smin, smin + 4):
        for tpc in range(base, base + 9):
            if s > tpc:
                continue

            # enumerate descending partitions of tpc into s parts
            def parts(tot, k, maxv):
                if k == 1:
                    if 1 <= tot <= maxv:
                        yield (tot,)
                    return
                for a in range(min(tot - k + 1, maxv), (tot - 1) // k, -1):
                    for rest in parts(tot - a, k - 1, a):
                        yield (a,) + rest

            for shape in parts(tpc, s, tpc):
                a = try_shape(shape)
                if a is None:
                    continue
                cost = tpc * (2 * P * D * 2) + s * (D * D * 2)
                if best is None or cost < best[0]:
                    best = (cost, shape, a)
    if best is None:
        # backstop: always feasible (each pair whole in one cell)
        shape = tuple([tmax] * (-(-ncells_min // NCORES)))
        best = (0, shape, try_shape(shape))
    _, sizes, assign = best
    return list(sizes), assign


def _build_program_raw(sizes):
    """Hand-scheduled raw-bass SPMD program (no TileContext): 5 engine
    streams with explicit semaphores. Avoids Tile's ~10us exit chain and
    gives exact control of DMA issue order / prefetch depth."""
    s = len(sizes)
    tpc = sum(sizes)
    tpcB = -(-tpc // B)
    lastB = tpc - (tpcB - 1) * B  # tiles in the last (possibly partial) super-tile
    bf16 = mybir.dt.bfloat16
    f32 = mybir.dt.float32
    XB, PB, WB = 6, 3, 5
    OB = -(-tpc // B)  # buffer ALL out super-tiles: stores are deferred

    # slot id for each global tile index
    slot_of = []
    for j, F in enumerate(sizes):
        slot_of.extend([j] * F)
    # tile index at which slot j ends
    slot_end = [0] * s
    for t, j in enumerate(slot_of):
        slot_end[j] = t + 1

    nc = bass.Bass("TRN2", target_bir_lowering=False, debug=False)
    xT = nc.dram_tensor("xT", [tpcB, P, B * D], bf16, kind="ExternalInput")
    w = nc.dram_tensor("w", [P, s * KC * D], bf16, kind="ExternalInput")
    out = nc.dram_tensor("out", [tpcB, P, B * D], bf16, kind="ExternalOutput")

    with ExitStack() as ctx:
        xb = [
            ctx.enter_context(nc.sbuf_tensor(f"xb{i}", [P, B * D], bf16))
            for i in range(XB)
        ]
        # ALL weight slots resident in one contiguous tensor: slots 1..s-1
        # arrive as two fused large DMAs (4MB/6MB run ~15-25% faster than
        # 2MB on the receipt-amortization curve), slot 0 stays chunked for
        # a fast head. No buffer rotation, no reuse gating.
        wbig = ctx.enter_context(
            nc.sbuf_tensor("wbig", [P, s * KC * D], bf16)
        )
        fuse_groups = []
        if s > 1:
            fuse_groups.append(list(range(1, min(3, s))))
        if s > 3:
            fuse_groups.append(list(range(3, s)))
        group_of = {j: g for g, grp in enumerate(fuse_groups) for j in grp}
        ob = [
            ctx.enter_context(nc.sbuf_tensor(f"ob{i}", [P, B * D], bf16))
            for i in range(OB)
        ]
        warm = ctx.enter_context(nc.sbuf_tensor("warmt", [P, 2 * P], bf16))
        pb = [
            ctx.enter_context(nc.psum_tensor(f"pb{i}", [P, D], f32))
            for i in range(PB)
        ]
        pwarm = ctx.enter_context(nc.psum_tensor("pwarm", [P, P], f32))
        # Completion semaphores: one DMA in flight per sem lane, so each
        # lane's cumulative count is sound. (A single cumulative sem across
        # several in-flight DMAs is NOT: 16 incs can come from 8 fast SDMA
        # engines x 2 transfers while slow engines still move transfer 0.)
        sem_x = [
            ctx.enter_context(nc.semaphore(f"sem_x{i}")) for i in range(XB)
        ]
        sem_wc = [
            ctx.enter_context(nc.semaphore(f"sem_wc{i}")) for i in range(KC)
        ]
        sem_wl = [
            ctx.enter_context(nc.semaphore(f"sem_wl{i}")) for i in range(2)
        ]
        sem_out = [
            ctx.enter_context(nc.semaphore(f"sem_o{i}")) for i in range(OB)
        ]
        sem_mm = ctx.enter_context(nc.semaphore("sem_mm"))
        sem_cp = ctx.enter_context(nc.semaphore("sem_cp"))

        # Prior programs (e.g. XLA executables) leave semaphores dirty.
        # Clear ours on gpsimd while every engine holds at an NRT-level
        # pseudo-barrier (safe before bass sems are valid), then start.
        sems = sem_x + sem_wc + sem_wl + sem_out + [sem_mm, sem_cp]
        nums = sorted(sm.num for sm in sems)
        nc.gpsimd.dma_reset(range(nums[0], nums[-1] + 1))
        nc._nrt_pseudo_barrier()

        block = ctx.enter_context(nc.Block())

        # Pace x loads ~AHEAD super-tiles in front of compute: issuing all
        # of them upfront steals early HBM bandwidth from the weight
        # stream, which is the DMA critical path and stalls the PE.
        AHEAD = 3

        # Slot 0 and the LAST slot arrive as WCH 0.5MB chunks with
        # per-chunk sem lanes (slot 0 on sem_wc[0:WCH], last slot on
        # sem_wc[WCH:2*WCH]) so the PE can chase chunks: at the head this
        # shortens time-to-first-matmul, and at the tail it overlaps the
        # final slot's transfer with its own compute instead of exposing
        # the whole 2MB latency after the stream ends.
        WCH = 4
        KPC = KC // WCH  # kk chunks per weight chunk


        # Stores are deferred until the final weight DMA has landed (the
        # weight stream is the DMA critical path; store traffic
        # interleaved with it delays kernel end by bytes/358GBps), then
        # split across the sync + gpsimd queues so the burst drains at 2x
        # the single-queue rate. All out super-tiles stay buffered in
        # SBUF (OB == tpcB, lane == tb), so copies never block on this.
        def emit_stores(eng, parity):
            # gate: final fused weight transfer has landed
            if fuse_groups:
                eng.wait_ge(sem_wl[len(fuse_groups) - 1], 16)
            else:
                eng.wait_ge(sem_wc[WCH - 1], 16)
            waits = []
            for tb in range(parity, tpcB - 1, 2):
                eng.wait_ge(sem_cp, 2 * B * (tb + 1))
                eng.dma_start(out=out[tb], in_=ob[tb][:]).then_inc(
                    sem_out[tb], 16
                )
                waits.append((tb, 1))
            # last super-tile: each engine takes one half-tile store, so
            # the two final stores run concurrently on the two queues.
            # BOTH engines then wait for the lane's combined count (an
            # engine's own >=16 could be satisfied by the other's incs).
            tb = tpcB - 1
            for b in range(lastB):
                if b % 2 != parity:
                    continue
                eng.wait_ge(sem_cp, 2 * (tb * B + b + 1))
                eng.dma_start(
                    out=out[tb][:, b * D : (b + 1) * D],
                    in_=ob[tb][:, b * D : (b + 1) * D],
                ).then_inc(sem_out[tb], 16)
            waits.append((tb, lastB))
            for tb, n in waits:
                eng.wait_ge(sem_out[tb], 16 * n)

        @block.sync
        def _(sync):
            for tb in range(tpcB):
                nb = lastB if tb == tpcB - 1 else B
                if tb >= AHEAD:
                    sync.wait_ge(sem_mm, min(B * (tb - AHEAD) + 1, tpc))
                sync.dma_start(
                    out=xb[tb % XB][:, : nb * D], in_=xT[tb][:, : nb * D]
                ).then_inc(sem_x[tb % XB], 16)
            emit_stores(sync, 1)

        @block.scalar
        def _(scalar):
            for q in range(WCH):
                scalar.dma_start(
                    out=wbig[:, q * KPC * D : (q + 1) * KPC * D],
                    in_=w[:, q * KPC * D : (q + 1) * KPC * D],
                ).then_inc(sem_wc[q], 16)
            for g, grp in enumerate(fuse_groups):
                lo, hi = grp[0] * KC * D, (grp[-1] + 1) * KC * D
                scalar.dma_start(out=wbig[:, lo:hi], in_=w[:, lo:hi]).then_inc(
                    sem_wl[g], 16
                )
            emit_stores(scalar, 0)

        @block.gpsimd
        def _(gpsimd):
            pass

        @block.tensor
        def _(tensor):
            # HAM warm-up on garbage SBUF (never read downstream): bf16
            # N=128 matmuls (~4.3us of PE-busy at the cold 1.2GHz clock)
            # bridging until slot 0's weights have fully landed. Length is
            # ~free: the kernel is DMA-bound, and entering the real tiles
            # gapless keeps HAM at 2.4GHz for the rest of the run (early
            # PE idle gaps > 3.4us trigger cold-clock spirals that CAN
            # fall behind the weight stream).
            for _ in range(54):
                nc.tensor.matmul(
                    pwarm[:], warm[:, :P], warm[:, P : 2 * P],
                    start=True, stop=True,
                )
            for t in range(tpc):
                tb, b = t // B, t % B
                j = slot_of[t]
                ps = pb[t % PB]
                if b == 0:
                    tensor.wait_ge(sem_x[tb % XB], 16 * (tb // XB + 1))
                if t >= PB:
                    tensor.wait_ge(sem_cp, 2 * (t - PB + 1))
                first_of_slot = t == 0 or slot_of[t - 1] != j
                if first_of_slot and j > 0:
                    tensor.wait_ge(sem_wl[group_of[j]], 16)
                for kk in range(KC):
                    if t == 0 and kk % KPC == 0:
                        tensor.wait_ge(sem_wc[kk // KPC], 16)
                    for nh in range(2):
                        mm = nc.tensor.matmul(
                            ps[:, nh * 512 : (nh + 1) * 512],
                            xb[tb % XB][:, b * D + kk * P : b * D + (kk + 1) * P],
                            wbig[:, j * KC * D + kk * D + nh * 512 : j * KC * D + kk * D + (nh + 1) * 512],
                            start=(kk == 0),
                            stop=(kk == KC - 1),
                        )
                mm.then_inc(sem_mm, 1)

        @block.vector
        def _(vector):
            for t in range(tpc):
                tb, b = t // B, t % B
                for nh in range(2):
                    if b == 0 and nh == 0:
                        vector.wait_ge(sem_mm, t + 1)
                        if tb >= OB:
                            vector.wait_ge(sem_out[tb % OB], 16 * (tb // OB))
                    elif nh == 0:
                        vector.wait_ge(sem_mm, t + 1)
                    nc.vector.tensor_copy(
                        ob[tb % OB][:, b * D + nh * 512 : b * D + (nh + 1) * 512],
                        pb[t % PB][:, nh * 512 : (nh + 1) * 512],
                    ).then_inc(sem_cp, 1)

    return nc


def _build_program(sizes):
    """Compile the SPMD Bass program for the given slot structure."""
    s = len(sizes)
    tpc = sum(sizes)
    tpcB = -(-tpc // B)  # super-tiles (last may contain a padding tile)
    tpad = tpcB * B
    nc = bacc.Bacc("TRN2", target_bir_lowering=False, debug=False)
    bf16 = mybir.dt.bfloat16
    f32 = mybir.dt.float32

    # slot id for each global tile index (padding tiles -> last slot)
    slot_of = []
    for j, F in enumerate(sizes):
        slot_of.extend([j] * F)
    slot_of.extend([s - 1] * (tpad - tpc))

    xT = nc.dram_tensor("xT", [tpcB, P, B * D], bf16, kind="ExternalInput")
    w = nc.dram_tensor("w", [s, P, KC * D], bf16, kind="ExternalInput")
    out = nc.dram_tensor("out", [tpcB, P, B * D], bf16, kind="ExternalOutput")

    with tile.TileContext(nc) as tc:
        with (
            tc.tile_pool(name="cp", bufs=1) as cp,
            tc.tile_pool(name="w0p", bufs=1) as w0p,
            tc.tile_pool(name="wp", bufs=2) as wp,
            tc.tile_pool(name="xp", bufs=4) as xp,
            tc.tile_pool(name="op", bufs=3) as op,
            tc.tile_pool(name="pp", bufs=3, space="PSUM") as pp,
            tc.tile_pool(name="wmp", bufs=1, space="PSUM") as wmp,
        ):
            # slot 0 chunked into KC pieces so the first matmul only waits
            # for 0.25MB of weights (fast head); later slots are single
            # 2MB transfers (peak DMA efficiency), double-buffered.
            wts0 = []
            for kk in range(KC):
                wt = w0p.tile([P, D], bf16, tag=f"w0_{kk}")
                nc.scalar.dma_start(out=wt[:], in_=w[0, :, kk * D : (kk + 1) * D])
                wts0.append(wt)
            # PE warm-up: small matmuls with no data deps run during the
            # initial DMA head, releasing the HAM clock gate (1.2->2.4GHz)
            # before the real matmuls start.
            wc = cp.tile([P, P + 32], f32, tag="warm")
            nc.gpsimd.memset(wc[:], 0.0)
            wps = wmp.tile([P, 32], f32, tag="warmps")
            for _ in range(14):
                nc.tensor.matmul(
                    wps[:], wc[:, :P], wc[:, P : P + 32], start=True, stop=True
                )
            wts = [None] * s
            for tb in range(tpcB):
                xt = xp.tile([P, B * D], bf16, tag="x")
                nc.sync.dma_start(out=xt[:], in_=xT[tb])
                ot = op.tile([P, B * D], bf16, tag="o")
                for b in range(B):
                    t = tb * B + b
                    j = slot_of[t]
                    if j > 0 and wts[j] is None:
                        wt = wp.tile([P, KC * D], bf16, tag="w")
                        nc.scalar.dma_start(out=wt[:], in_=w[j])
                        wts[j] = wt
                    # prefetch the next slot at slot entry: its DMA starts
                    # as soon as the j-1 buffer frees, overlapping slot j's
                    # matmuls
                    jn = j + 1
                    if jn < s and wts[jn] is None:
                        wt = wp.tile([P, KC * D], bf16, tag="w")
                        nc.scalar.dma_start(out=wt[:], in_=w[jn])
                        wts[jn] = wt
                    ps = pp.tile([P, D], f32, tag="ps")
                    for kk in range(KC):
                        lhsT = xt[:, b * D + kk * P : b * D + (kk + 1) * P]
                        for nh in range(2):
                            if j == 0:
                                rhs = wts0[kk][:, nh * 512 : (nh + 1) * 512]
                            else:
                                rhs = wts[j][
                                    :, kk * D + nh * 512 : kk * D + (nh + 1) * 512
                                ]
                            nc.tensor.matmul(
                                ps[:, nh * 512 : (nh + 1) * 512],
                                lhsT,
                                rhs,
                                start=(kk == 0),
                                stop=(kk == KC - 1),
                            )
                    for nh in range(2):
                        nc.vector.tensor_copy(
                            ot[:, b * D + nh * 512 : b * D + (nh + 1) * 512],
                            ps[:, nh * 512 : (nh + 1) * 512],
                        )
                nc.gpsimd.dma_start(out=out[tb], in_=ot[:])
    nc.compile()
    return nc


def kernel(x, Wr, br, We, be):
    global _last_results
    x = np.ascontiguousarray(np.asarray(x, dtype=np.float32))
    Wr = np.asarray(Wr, dtype=np.float32)
    br = np.asarray(br, dtype=np.float32)
    We = np.asarray(We, dtype=np.float32)
    be = np.asarray(be, dtype=np.float32)

    idx = _route(x, Wr, br)  # [N, 2] int32

    # canonical pair id per token: (a<b) -> a*E - a*(a+1)/2 + (b-a-1)
    a = np.minimum(idx[:, 0], idx[:, 1]).astype(np.int64)
    b = np.maximum(idx[:, 0], idx[:, 1]).astype(np.int64)
    pid = a * E - a * (a + 1) // 2 + (b - a - 1)
    pair_ab = [(ai, bi) for ai in range(E) for bi in range(ai + 1, E)]

    token_lists = [np.nonzero(pid == p)[0] for p in range(NPAIR)]
    tiles_per_pair = [-(-len(t) // P) for t in token_lists]
    sizes, assign = _pack(tiles_per_pair)
    s, tpc = len(sizes), sum(sizes)
    tpcB = -(-tpc // B)
    tpad = tpcB * B
    slot_off = np.concatenate([[0], np.cumsum(sizes)])  # tile offset of slot j

    # Build per-core inputs + bookkeeping
    bf = ml_dtypes.bfloat16
    xT_cores = np.zeros((NCORES, tpcB, P, B * D), dtype=bf)
    w_cores = np.zeros((NCORES, P, s * KC * D), dtype=bf)
    pos = np.full(N, -1, dtype=np.int64)
    cnt = np.zeros(N, dtype=np.int64)

    taken = [0] * NPAIR  # tiles of pair p already dispatched
    for p, c, j, ntiles in assign:
        toks_all = token_lists[p]
        start = taken[p] * P
        stop = min(start + ntiles * P, len(toks_all))
        taken[p] += ntiles
        toks = toks_all[start:stop]
        nrow = len(toks)
        ea, eb = pair_ab[p]
        # pair weights for this cell's slot: [ki, kk*D + n] = Wp[kk*P+ki, n]
        Wp = 0.5 * (We[ea] + We[eb])
        w_cores[c, :, j * KC * D : (j + 1) * KC * D] = (
            Wp.reshape(KC, P, D).transpose(1, 0, 2).reshape(P, KC * D).astype(bf)
        )
        if nrow == 0:
            continue
        nt_used = -(-nrow // P)
        xs = np.zeros((nt_used * P, D), dtype=np.float32)
        xs[:nrow] = x[toks]
        # per tile: [ki, kk*P + m] = xs[tile*P + m, kk*P + ki]
        blk = xs.reshape(nt_used, P, KC, P).transpose(0, 3, 2, 1).reshape(
            nt_used, P, D
        )
        t0 = slot_off[j]
        for ti in range(nt_used):
            t = t0 + ti
            xT_cores[c, t // B, :, (t % B) * D : (t % B + 1) * D] = blk[ti].astype(bf)
        # flat row positions in the concatenated [NCORES * tpad * P] output
        flat = c * (tpad * P) + t0 * P + np.arange(nrow)
        pos[toks] = flat
        cnt[toks] += 1

    assert (cnt == 1).all(), "dispatch did not cover every token exactly once"

    if os.environ.get("KERNEL_IMPL", "raw") == "raw":
        nc = _build_program_raw(sizes)
    else:
        nc = _build_program(sizes)
    in_maps = [{"xT": xT_cores[c], "w": w_cores[c]} for c in range(NCORES)]
    res = run_bass_kernel_spmd(nc, in_maps, core_ids=list(range(NCORES)))
    _last_results = res

    # out dram [tpcB, P, B*D] -> token-major rows [tpad*P, D]
    partial = np.concatenate(
        [
            np.asarray(res.results[c]["out"], dtype=np.float32)
            .reshape(tpcB, P, B, D)
            .transpose(0, 2, 1, 3)
            .reshape(tpad * P, D)
            for c in range(NCORES)
        ]
    )
    out = partial[pos] + 0.5 * (be[idx[:, 0]] + be[idx[:, 1]])
    return out.astype(np.float32)


# revision 70
# speedup vs baseline: 1.1730x; 1.1730x over previous
"""MoE top-2 routing kernel for Trainium2 (8 NeuronCores, pair-dispatch).

Key algebraic trick: the reference combine is an UNWEIGHTED mean, so
  out = 0.5*(x@Wa + be_a + x@Wb + be_b) = x @ (0.5*(Wa+Wb)) + 0.5*(be_a+be_b).
Dispatching tokens by their (unordered) top-2 expert PAIR and pre-summing
the 45 possible pair matrices on the host means ONE matmul per token
instead of two: half the PE streams and half the x/out HBM traffic of a
per-expert dispatch.

All device traffic is bf16 (x, pair weights, outputs; fp32 PSUM
accumulate), halving HBM bytes again. Measured rel-err of the full
pipeline ~3e-3 vs the fp32 reference (gate: 2e-2).

Layout per core (SPMD — all 8 cores run one compiled program):
  - `s` weight slots, slot j sized F_j 128-token tiles; each slot holds
    one pair matrix [1024,1024] bf16 = 2MB, DMA'd in a single transfer.
  - token tiles packed in super-tiles of B=2 tiles -> 0.5MB x loads /
    out stores for DMA efficiency.
  - matmul: x tile chunk [128k,128m] stationary, pair weights streamed
    [128k, 512n] x2 into a [128,1024] fp32 PSUM; DVE copies+casts to a
    bf16 out super-tile.
Host combines: out[token] = partial[pos[token]] + 0.5*(be_a + be_b).
"""

import os
from contextlib import ExitStack

import ml_dtypes
import numpy as np

import concourse.bass as bass
import concourse.mybir as mybir
import concourse.tile as tile
from concourse import bacc
from concourse.bass_utils import run_bass_kernel_spmd

N = 8192
D = 1024
E = 10
TOP_K = 2
P = 128
KC = D // P  # 8 contraction chunks of 128
NCORES = 8
B = 2  # token tiles per super-tile (DMA batch)
NPAIR = E * (E - 1) // 2  # 45

_last_results = None  # stash for test harness (exec_time_ns etc.)


def _route(x, Wr, br):
    """Top-2 expert ids per token, replicating reference ops exactly."""
    import jax
    import jax.numpy as jnp

    logits = jnp.asarray(x) @ jnp.asarray(Wr).T + jnp.asarray(br)
    probs = jax.nn.softmax(logits, axis=-1)
    _, idx = jax.lax.top_k(probs, TOP_K)
    return np.asarray(idx)


def _pack(tiles_per_pair):
    """Choose slot sizes (shared by all cores, SPMD) and assign pair
    tile-pieces to (core, slot) cells. Returns (sizes, assign): assign is
    a list of (pair, core, slot, n_tiles)."""
    active = [(t, e) for e, t in enumerate(tiles_per_pair) if t > 0]
    total = sum(t for t, _ in active)
    tmax = max(t for t, _ in active)
    ncells_min = len(active)  # each piece needs its own cell

    def try_shape(sizes):
        cells = sorted(
            [(sz, c, j) for j, sz in enumerate(sizes) for c in range(NCORES)],
            reverse=True,
        )
        rem = sorted(active, reverse=True)
        assign = []
        ci = 0
        while rem:
            rem.sort(reverse=True)
            t, e = rem.pop(0)
            if ci >= len(cells):
                return None
            sz, c, j = cells[ci]
            ci += 1
            if sz == 0:
                return None
            take = min(t, sz)
            assign.append((e, c, j, take))
            if t - take > 0:
                rem.append((t - take, e))
        return assign

    best = None
    base = -(-total // NCORES)  # ceil
    smin = max(1, -(-ncells_min // NCORES))
    for s in range(smin, smin + 4):
        for tpc in range(base, base + 9):
            if s > tpc:
                continue

            # enumerate descending partitions of tpc into s parts
            def parts(tot, k, maxv):
                if k == 1:
                    if 1 <= tot <= maxv:
                        yield (tot,)
                    return
                for a in range(min(tot - k + 1, maxv), (tot - 1) // k, -1):
                    for rest in parts(tot - a, k - 1, a):
                        yield (a,) + rest

            for shape in parts(tpc, s, tpc):
                a = try_shape(shape)
                if a is None:
                    continue
                cost = tpc * (2 * P * D * 2) + s * (D * D * 2)
                if best is None or cost < best[0]:
                    best = (cost, shape, a)
    if best is None:
        # backstop: always feasible (each pair whole in one cell)
        shape = tuple([tmax] * (-(-ncells_min // NCORES)))
        best = (0, shape, try_shape(shape))
    _, sizes, assign = best
    return list(sizes), assign


def _build_program_raw(sizes):
    """Hand-scheduled raw-bass SPMD program (no TileContext): 5 engine
    streams with explicit semaphores. Avoids Tile's ~10us exit chain and
    gives exact control of DMA issue order / prefetch depth."""
    s = len(sizes)
    tpc = sum(sizes)
    tpcB = -(-tpc // B)
    lastB = tpc - (tpcB - 1) * B  # tiles in the last (possibly partial) super-tile
    bf16 = mybir.dt.bfloat16
    f32 = mybir.dt.float32
    XB, PB, WB = 6, 3, 5
    OB = -(-tpc // B)  # buffer ALL out super-tiles: stores are deferred

    # slot id for each global tile index
    slot_of = []
    for j, F in enumerate(sizes):
        slot_of.extend([j] * F)
    # tile index at which slot j ends
    slot_end = [0] * s
    for t, j in enumerate(slot_of):
        slot_end[j] = t + 1

    nc = bass.Bass("TRN2", target_bir_lowering=False, debug=False)
    xT = nc.dram_tensor("xT", [tpcB, P, B * D], bf16, kind="ExternalInput")
    w = nc.dram_tensor("w", [s, P, KC * D], bf16, kind="ExternalInput")
    out = nc.dram_tensor("out", [tpcB, P, B * D], bf16, kind="ExternalOutput")

    with ExitStack() as ctx:
        xb = [
            ctx.enter_context(nc.sbuf_tensor(f"xb{i}", [P, B * D], bf16))
            for i in range(XB)
        ]
        wb = [
            ctx.enter_context(nc.sbuf_tensor(f"wb{i}", [P, KC * D], bf16))
            for i in range(WB)
        ]
        ob = [
            ctx.enter_context(nc.sbuf_tensor(f"ob{i}", [P, B * D], bf16))
            for i in range(OB)
        ]
        warm = ctx.enter_context(nc.sbuf_tensor("warmt", [P, 2 * P], bf16))
        pb = [
            ctx.enter_context(nc.psum_tensor(f"pb{i}", [P, D], f32))
            for i in range(PB)
        ]
        pwarm = ctx.enter_context(nc.psum_tensor("pwarm", [P, P], f32))
        # Completion semaphores: one DMA in flight per sem lane, so each
        # lane's cumulative count is sound. (A single cumulative sem across
        # several in-flight DMAs is NOT: 16 incs can come from 8 fast SDMA
        # engines x 2 transfers while slow engines still move transfer 0.)
        sem_x = [
            ctx.enter_context(nc.semaphore(f"sem_x{i}")) for i in range(XB)
        ]
        sem_wc = [
            ctx.enter_context(nc.semaphore(f"sem_wc{i}")) for i in range(KC)
        ]
        sem_wl = [
            ctx.enter_context(nc.semaphore(f"sem_wl{i}")) for i in range(WB)
        ]
        sem_out = [
            ctx.enter_context(nc.semaphore(f"sem_o{i}")) for i in range(OB)
        ]
        sem_mm = ctx.enter_context(nc.semaphore("sem_mm"))
        sem_cp = ctx.enter_context(nc.semaphore("sem_cp"))

        # Prior programs (e.g. XLA executables) leave semaphores dirty.
        # Clear ours on gpsimd while every engine holds at an NRT-level
        # pseudo-barrier (safe before bass sems are valid), then start.
        sems = sem_x + sem_wc + sem_wl + sem_out + [sem_mm, sem_cp]
        nums = sorted(sm.num for sm in sems)
        nc.gpsimd.dma_reset(range(nums[0], nums[-1] + 1))
        nc._nrt_pseudo_barrier()

        block = ctx.enter_context(nc.Block())

        # Pace x loads ~AHEAD super-tiles in front of compute: issuing all
        # of them upfront steals early HBM bandwidth from the weight
        # stream, which is the DMA critical path and stalls the PE.
        AHEAD = 3

        # Slot 0 and the LAST slot arrive as WCH 0.5MB chunks with
        # per-chunk sem lanes (slot 0 on sem_wc[0:WCH], last slot on
        # sem_wc[WCH:2*WCH]) so the PE can chase chunks: at the head this
        # shortens time-to-first-matmul, and at the tail it overlaps the
        # final slot's transfer with its own compute instead of exposing
        # the whole 2MB latency after the stream ends.
        WCH = 4
        KPC = KC // WCH  # kk chunks per weight chunk
        # nth weight DMA on lane j%WB for slots 1..s-1
        lane_n = [0] * WB
        wl_count = [0] * s
        for j in range(1, s):
            lane_n[j % WB] += 1
            wl_count[j] = lane_n[j % WB]

        # Stores are deferred until the final weight DMA has landed (the
        # weight stream is the DMA critical path; store traffic
        # interleaved with it delays kernel end by bytes/358GBps), then
        # split across the sync + gpsimd queues so the burst drains at 2x
        # the single-queue rate. All out super-tiles stay buffered in
        # SBUF (OB == tpcB, lane == tb), so copies never block on this.
        def emit_stores(eng, parity):
            # gate: final weight slot has landed
            if s > 1:
                eng.wait_ge(sem_wl[(s - 1) % WB], 16 * wl_count[s - 1])
            else:
                eng.wait_ge(sem_wc[WCH - 1], 16)
            waits = []
            for tb in range(parity, tpcB - 1, 2):
                eng.wait_ge(sem_cp, 2 * B * (tb + 1))
                eng.dma_start(out=out[tb], in_=ob[tb][:]).then_inc(
                    sem_out[tb], 16
                )
                waits.append((tb, 1))
            # last super-tile: each engine takes one half-tile store, so
            # the two final stores run concurrently on the two queues.
            # BOTH engines then wait for the lane's combined count (an
            # engine's own >=16 could be satisfied by the other's incs).
            tb = tpcB - 1
            for b in range(lastB):
                if b % 2 != parity:
                    continue
                eng.wait_ge(sem_cp, 2 * (tb * B + b + 1))
                eng.dma_start(
                    out=out[tb][:, b * D : (b + 1) * D],
                    in_=ob[tb][:, b * D : (b + 1) * D],
                ).then_inc(sem_out[tb], 16)
            waits.append((tb, lastB))
            for tb, n in waits:
                eng.wait_ge(sem_out[tb], 16 * n)

        @block.sync
        def _(sync):
            for tb in range(tpcB):
                nb = lastB if tb == tpcB - 1 else B
                if tb >= AHEAD:
                    sync.wait_ge(sem_mm, min(B * (tb - AHEAD) + 1, tpc))
                sync.dma_start(
                    out=xb[tb % XB][:, : nb * D], in_=xT[tb][:, : nb * D]
                ).then_inc(sem_x[tb % XB], 16)
            emit_stores(sync, 1)

        @block.scalar
        def _(scalar):
            for q in range(WCH):
                scalar.dma_start(
                    out=wb[0][:, q * KPC * D : (q + 1) * KPC * D],
                    in_=w[0, :, q * KPC * D : (q + 1) * KPC * D],
                ).then_inc(sem_wc[q], 16)
            for j in range(1, s):
                if j >= WB:
                    scalar.wait_ge(sem_mm, slot_end[j - WB])
                scalar.dma_start(out=wb[j % WB][:], in_=w[j]).then_inc(
                    sem_wl[j % WB], 16
                )
            emit_stores(scalar, 0)

        @block.gpsimd
        def _(gpsimd):
            pass

        @block.tensor
        def _(tensor):
            # HAM warm-up on garbage SBUF (never read downstream): bf16
            # N=128 matmuls (~4.3us of PE-busy at the cold 1.2GHz clock)
            # bridging until slot 0's weights have fully landed. Length is
            # ~free: the kernel is DMA-bound, and entering the real tiles
            # gapless keeps HAM at 2.4GHz for the rest of the run (early
            # PE idle gaps > 3.4us trigger cold-clock spirals that CAN
            # fall behind the weight stream).
            for _ in range(54):
                nc.tensor.matmul(
                    pwarm[:], warm[:, :P], warm[:, P : 2 * P],
                    start=True, stop=True,
                )
            for t in range(tpc):
                tb, b = t // B, t % B
                j = slot_of[t]
                ps = pb[t % PB]
                if b == 0:
                    tensor.wait_ge(sem_x[tb % XB], 16 * (tb // XB + 1))
                if t >= PB:
                    tensor.wait_ge(sem_cp, 2 * (t - PB + 1))
                first_of_slot = t == 0 or slot_of[t - 1] != j
                if first_of_slot and j > 0:
                    tensor.wait_ge(sem_wl[j % WB], 16 * wl_count[j])
                for kk in range(KC):
                    if t == 0 and kk % KPC == 0:
                        tensor.wait_ge(sem_wc[kk // KPC], 16)
                    for nh in range(2):
                        mm = nc.tensor.matmul(
                            ps[:, nh * 512 : (nh + 1) * 512],
                            xb[tb % XB][:, b * D + kk * P : b * D + (kk + 1) * P],
                            wb[j % WB][:, kk * D + nh * 512 : kk * D + (nh + 1) * 512],
                            start=(kk == 0),
                            stop=(kk == KC - 1),
                        )
                mm.then_inc(sem_mm, 1)

        @block.vector
        def _(vector):
            for t in range(tpc):
                tb, b = t // B, t % B
                for nh in range(2):
                    if b == 0 and nh == 0:
                        vector.wait_ge(sem_mm, t + 1)
                        if tb >= OB:
                            vector.wait_ge(sem_out[tb % OB], 16 * (tb // OB))
                    elif nh == 0:
                        vector.wait_ge(sem_mm, t + 1)
                    nc.vector.tensor_copy(
                        ob[tb % OB][:, b * D + nh * 512 : b * D + (nh + 1) * 512],
                        pb[t % PB][:, nh * 512 : (nh + 1) * 512],
                    ).then_inc(sem_cp, 1)

    return nc


def _build_program(sizes):
    """Compile the SPMD Bass program for the given slot structure."""
    s = len(sizes)
    tpc = sum(sizes)
    tpcB = -(-tpc // B)  # super-tiles (last may contain a padding tile)
    tpad = tpcB * B
    nc = bacc.Bacc("TRN2", target_bir_lowering=False, debug=False)
    bf16 = mybir.dt.bfloat16
    f32 = mybir.dt.float32

    # slot id for each global tile index (padding tiles -> last slot)
    slot_of = []
    for j, F in enumerate(sizes):
        slot_of.extend([j] * F)
    slot_of.extend([s - 1] * (tpad - tpc))

    xT = nc.dram_tensor("xT", [tpcB, P, B * D], bf16, kind="ExternalInput")
    w = nc.dram_tensor("w", [s, P, KC * D], bf16, kind="ExternalInput")
    out = nc.dram_tensor("out", [tpcB, P, B * D], bf16, kind="ExternalOutput")

    with tile.TileContext(nc) as tc:
        with (
            tc.tile_pool(name="cp", bufs=1) as cp,
            tc.tile_pool(name="w0p", bufs=1) as w0p,
            tc.tile_pool(name="wp", bufs=2) as wp,
            tc.tile_pool(name="xp", bufs=4) as xp,
            tc.tile_pool(name="op", bufs=3) as op,
            tc.tile_pool(name="pp", bufs=3, space="PSUM") as pp,
            tc.tile_pool(name="wmp", bufs=1, space="PSUM") as wmp,
        ):
            # slot 0 chunked into KC pieces so the first matmul only waits
            # for 0.25MB of weights (fast head); later slots are single
            # 2MB transfers (peak DMA efficiency), double-buffered.
            wts0 = []
            for kk in range(KC):
                wt = w0p.tile([P, D], bf16, tag=f"w0_{kk}")
                nc.scalar.dma_start(out=wt[:], in_=w[0, :, kk * D : (kk + 1) * D])
                wts0.append(wt)
            # PE warm-up: small matmuls with no data deps run during the
            # initial DMA head, releasing the HAM clock gate (1.2->2.4GHz)
            # before the real matmuls start.
            wc = cp.tile([P, P + 32], f32, tag="warm")
            nc.gpsimd.memset(wc[:], 0.0)
            wps = wmp.tile([P, 32], f32, tag="warmps")
            for _ in range(14):
                nc.tensor.matmul(
                    wps[:], wc[:, :P], wc[:, P : P + 32], start=True, stop=True
                )
            wts = [None] * s
            for tb in range(tpcB):
                xt = xp.tile([P, B * D], bf16, tag="x")
                nc.sync.dma_start(out=xt[:], in_=xT[tb])
                ot = op.tile([P, B * D], bf16, tag="o")
                for b in range(B):
                    t = tb * B + b
                    j = slot_of[t]
                    if j > 0 and wts[j] is None:
                        wt = wp.tile([P, KC * D], bf16, tag="w")
                        nc.scalar.dma_start(out=wt[:], in_=w[j])
                        wts[j] = wt
                    # prefetch the next slot at slot entry: its DMA starts
                    # as soon as the j-1 buffer frees, overlapping slot j's
                    # matmuls
                    jn = j + 1
                    if jn < s and wts[jn] is None:
                        wt = wp.tile([P, KC * D], bf16, tag="w")
                        nc.scalar.dma_start(out=wt[:], in_=w[jn])
                        wts[jn] = wt
                    ps = pp.tile([P, D], f32, tag="ps")
                    for kk in range(KC):
                        lhsT = xt[:, b * D + kk * P : b * D + (kk + 1) * P]
                        for nh in range(2):
                            if j == 0:
                                rhs = wts0[kk][:, nh * 512 : (nh + 1) * 512]
                            else:
                                rhs = wts[j][
                                    :, kk * D + nh * 512 : kk * D + (nh + 1) * 512
                                ]
                            nc.tensor.matmul(
                                ps[:, nh * 512 : (nh + 1) * 512],
                                lhsT,
                                rhs,
                                start=(kk == 0),
                                stop=(kk == KC - 1),
                            )
                    for nh in range(2):
                        nc.vector.tensor_copy(
                            ot[:, b * D + nh * 512 : b * D + (nh + 1) * 512],
                            ps[:, nh * 512 : (nh + 1) * 512],
                        )
                nc.gpsimd.dma_start(out=out[tb], in_=ot[:])
    nc.compile()
    return nc


def kernel(x, Wr, br, We, be):
    global _last_results
    x = np.ascontiguousarray(np.asarray(x, dtype=np.float32))
    Wr = np.asarray(Wr, dtype=np.float32)
    br = np.asarray(br, dtype=np.float32)
    We = np.asarray(We, dtype=np.float32)
    be = np.asarray(be, dtype=np.float32)

    idx = _route(x, Wr, br)  # [N, 2] int32

    # canonical pair id per token: (a<b) -> a*E - a*(a+1)/2 + (b-a-1)
    a = np.minimum(idx[:, 0], idx[:, 1]).astype(np.int64)
    b = np.maximum(idx[:, 0], idx[:, 1]).astype(np.int64)
    pid = a * E - a * (a + 1) // 2 + (b - a - 1)
    pair_ab = [(ai, bi) for ai in range(E) for bi in range(ai + 1, E)]

    token_lists = [np.nonzero(pid == p)[0] for p in range(NPAIR)]
    tiles_per_pair = [-(-len(t) // P) for t in token_lists]
    sizes, assign = _pack(tiles_per_pair)
    s, tpc = len(sizes), sum(sizes)
    tpcB = -(-tpc // B)
    tpad = tpcB * B
    slot_off = np.concatenate([[0], np.cumsum(sizes)])  # tile offset of slot j

    # Build per-core inputs + bookkeeping
    bf = ml_dtypes.bfloat16
    xT_cores = np.zeros((NCORES, tpcB, P, B * D), dtype=bf)
    w_cores = np.zeros((NCORES, s, P, KC * D), dtype=bf)
    pos = np.full(N, -1, dtype=np.int64)
    cnt = np.zeros(N, dtype=np.int64)

    taken = [0] * NPAIR  # tiles of pair p already dispatched
    for p, c, j, ntiles in assign:
        toks_all = token_lists[p]
        start = taken[p] * P
        stop = min(start + ntiles * P, len(toks_all))
        taken[p] += ntiles
        toks = toks_all[start:stop]
        nrow = len(toks)
        ea, eb = pair_ab[p]
        # pair weights for this cell's slot: [ki, kk*D + n] = Wp[kk*P+ki, n]
        Wp = 0.5 * (We[ea] + We[eb])
        w_cores[c, j] = (
            Wp.reshape(KC, P, D).transpose(1, 0, 2).reshape(P, KC * D).astype(bf)
        )
        if nrow == 0:
            continue
        nt_used = -(-nrow // P)
        xs = np.zeros((nt_used * P, D), dtype=np.float32)
        xs[:nrow] = x[toks]
        # per tile: [ki, kk*P + m] = xs[tile*P + m, kk*P + ki]
        blk = xs.reshape(nt_used, P, KC, P).transpose(0, 3, 2, 1).reshape(
            nt_used, P, D
        )
        t0 = slot_off[j]
        for ti in range(nt_used):
            t = t0 + ti
            xT_cores[c, t // B, :, (t % B) * D : (t % B + 1) * D] = blk[ti].astype(bf)
        # flat row positions in the concatenated [NCORES * tpad * P] output
        flat = c * (tpad * P) + t0 * P + np.arange(nrow)
        pos[toks] = flat
        cnt[toks] += 1

    assert (cnt == 1).all(), "dispatch did not cover every token exactly once"

    if os.environ.get("KERNEL_IMPL", "raw") == "raw":
        nc = _build_program_raw(sizes)
    else:
        nc = _build_program(sizes)
    in_maps = [{"xT": xT_cores[c], "w": w_cores[c]} for c in range(NCORES)]
    res = run_bass_kernel_spmd(nc, in_maps, core_ids=list(range(NCORES)))
    _last_results = res

    # out dram [tpcB, P, B*D] -> token-major rows [tpad*P, D]
    partial = np.concatenate(
        [
            np.asarray(res.results[c]["out"], dtype=np.float32)
            .reshape(tpcB, P, B, D)
            .transpose(0, 2, 1, 3)
            .reshape(tpad * P, D)
            for c in range(NCORES)
        ]
    )
    out = partial[pos] + 0.5 * (be[idx[:, 0]] + be[idx[:, 1]])
    return out.astype(np.float32)
